# revision 37
# baseline (speedup 1.0000x reference)
"""EmergentSpinGlass fused kernel for 8 Trainium2 NeuronCores.

Reference computation (per batch b):
    s   = x @ W_spin.T + b_spin                       (N, D)
    mf  = mean_n s                                    (D,)
    g   = W_global @ mf                               (D,)   [same for all rows]
    EF  = s @ W_J.T                                   (N, D)
    A   = softmax(EF @ s.T / sqrt(D), axis=-1)        (N, N)
    LF  = A @ s                                       (N, D)
    out = tanh(beta * (s + g + LF))                   (N, D)

Sharding: 8 cores = 4 batches x 2 query-halves. Each core receives x^T for
its batch with its query half's rows permuted first (attention is
permutation-invariant over keys), computes s for all 2048 keys, and runs
the attention block for its 1024 queries. Weights are pre-transposed on
the host; all device matmuls contract over the SBUF partition dim.

Dtype strategy (validated numerically: end-to-end rel err ~3.2e-3 vs the
2e-2 gate):
  - own-half s (the query rows) in bf16: feeds the dominant s-term.
  - remote-half s is only ever consumed as fp8 (scores rhs, attention
    values, mean-field), so it is computed straight in fp8e4 DoubleRow
    from fp8 x / fp8 32*W_spin at ~1.8x bf16 PE rate; the ACT epilogue
    rescales by 1/32, adds bias, and accumulates the mean in one op.
  - all attention matmuls in fp8e4 (e4m3, max 240) with
    perf_mode=DoubleRow: each instruction contracts TWO 128-k-tiles
    (operands [K,2,M]/[K,2,N]). Softmax weight noise (~4%) averages out
    over 2048 keys.
  - W_J and W_spin(fp8 copy) pre-scaled x32 on host (raw ~1/32 values are
    subnormal in e4m3); exp() scale folds 1/32 back in.
  - scores are computed TRANSPOSED ([key, query] layout, 512-query
    groups): exp writes fp8 P^T directly (unnormalized, values in e4m3
    normal range), so no P transposes, no P normalization pass. Rowsums
    come from a ones-vector DoubleRow matmul; 1/rowsum is applied to the
    PSUM local-field rows (queries on partitions) in the epilogue, scaled
    x256 against the 256*s term, with tanh scale beta/256. The g term
    rides inside the local-field PSUM group as a rank-1 matmul
    rowsum[q] x g[d], which the later rinv multiply turns into exactly g.
  - output stored bf16 (values in [-1,1]), upcast on host.

Softmax skips the running-max subtraction: scaled scores for this
problem's distribution are bounded (|scores|/sqrt(D) < ~2 with huge
margin), so exp() cannot overflow; softmax itself is shift-invariant.

Structure, tuned from hardware profiles:
  - x^T streams in 512-key chunks; the first bf16 chunk's DMA is split by
    contraction tiles so the first matmul pass waits on ~0.75MB only.
  - phase 1a (own half, bf16 matmuls) writes s^T twice from PSUM: DVE
    adds bias -> bf16 ST (+ mean accum), ACT adds bias -> fp8 ST8.
  - phase 1b (remote half) runs fp8 DoubleRow matmuls; ACT rescales,
    adds bias, writes ST8 and accumulates the mean — DVE untouched.
  - attention is two 512-query groups, each: 64 scores^T matmuls ->
    16 exp -> 8 rowsum matmuls -> 4x local-field/epilogue; group 1's
    scores overlap group 0's epilogues.
  - PE transposes write 4 tiles into one PSUM bank before a single
    512-wide copy (copy cost is latency-dominated).
"""

import numpy as np
import ml_dtypes

import concourse.bass as bass
import concourse.tile as tile
from concourse import bacc, mybir
from concourse import bass_utils
from concourse.masks import make_identity
from concourse.bass_interp import get_hw_module

F32 = mybir.dt.float32
BF16 = mybir.dt.bfloat16
F8 = mybir.dt.float8e4
ADD = mybir.AluOpType.add
DR = mybir.MatmulPerfMode.DoubleRow
IDENT = mybir.ActivationFunctionType.Identity

B, N, D = 4, 2048, 1024
NQ = N // 2          # queries per core (= own keys)
KT = D // 128        # 8 contraction tiles
KP = KT // 2         # 4 DoubleRow pair-tiles
MT = N // 128        # 16 key tiles
QT = NQ // 128       # 8 query tiles
SCALE = 1.0 / np.sqrt(np.float32(D))
WJ_SCALE = 32.0      # host pre-scale of W_J / fp8 W_spin
P_SCALE = 256.0      # scale on normalized softmax weights vs the s term

LAST_RESULT = None   # BassKernelResults of the most recent run (for test.py)
_CACHED = {}


def _build(debug=False):
    nc = bacc.Bacc(
        "TRN2",
        target_bir_lowering=False,
        debug=False,
        enable_asserts=False,
        num_devices=8,
    )
    xtb_d = nc.dram_tensor("xtb", [128, KT, NQ], BF16, kind="ExternalInput").ap()
    xt8_d = nc.dram_tensor("xt8", [128, KT, NQ], F8, kind="ExternalInput").ap()
    wspin_d = nc.dram_tensor("wspinT", [128, KT, D], BF16, kind="ExternalInput").ap()
    wspin8_d = nc.dram_tensor("wspin8T", [128, KT, D], F8, kind="ExternalInput").ap()
    wj_d = nc.dram_tensor("wjT", [128, KT, D], F8, kind="ExternalInput").ap()
    wglob_d = nc.dram_tensor("wglobT", [128, KT, D], F8, kind="ExternalInput").ap()
    bspin_d = nc.dram_tensor("bspin", [128, KT], F32, kind="ExternalInput").ap()
    beta_d = nc.dram_tensor("beta", [1, 1], F32, kind="ExternalInput").ap()
    out_d = nc.dram_tensor("out", [NQ, D], BF16, kind="ExternalOutput").ap()

    with tile.TileContext(nc) as tc:
        with (
            tc.tile_pool(name="const", bufs=1) as const,
            tc.tile_pool(name="longp", bufs=1) as longp,
            tc.tile_pool(name="stats", bufs=8) as stats,
        ):
            ident32 = const.tile([128, 128], F32)
            make_identity(nc, ident32)
            ident_b = const.tile([128, 128], BF16)
            nc.vector.tensor_copy(ident_b[:], ident32[:])
            ident_8 = const.tile([128, 128], F8)
            nc.vector.tensor_copy(ident_8[:], ident32[:])
            # fp8 pair-of-ones for rowsum DoubleRow matmuls; [2,16] so the
            # pair stride is 16B (DR requires 16-aligned pair step)
            ones8 = const.tile([128, 2, 16], F8)
            nc.vector.memset(ones8, 1.0)
            c_inv256 = const.tile([1, 1], BF16)
            nc.vector.memset(c_inv256, 1.0 / P_SCALE)
            beta_sb = const.tile([128, 1], F32)
            nc.gpsimd.dma_start(out=beta_sb[:], in_=beta_d.to_broadcast((128, 1)))
            beta256 = const.tile([128, 1], F32)
            nc.vector.tensor_scalar_mul(beta256[:], beta_sb[:], 1.0 / P_SCALE)
            bspin_sb = const.tile([128, KT], F32)
            nc.gpsimd.dma_start(out=bspin_sb[:], in_=bspin_d[:])
            mf4 = const.tile([128, KT, 4], F32)
            mf = const.tile([128, KT], F32)
            # 32*mf in fp8, strided 16B so DoubleRow pair slices are legal
            mfs = const.tile([128, KT, 16], F8)
            gT = const.tile([1, D], BF16)

            ST = longp.tile([128, KT, NQ], BF16)   # own-half s^T (bf16)
            ST8 = longp.tile([128, KT, N], F8)     # all keys' s^T (fp8)
            SN = longp.tile([128, MT, D], F8)      # [key-in-tile, key-tile, d]
            SQ256 = longp.tile([128, QT, D], BF16)  # 256 * s, query rows

            # pool stack: efp {EF8} > wjp {wj8} > ph1 {x chunks, wspin}
            efp_cm = tc.tile_pool(name="efp", bufs=1)
            efp = efp_cm.__enter__()
            wjp_cm = tc.tile_pool(name="wjp", bufs=1)
            wjp = wjp_cm.__enter__()

            # ---- Phase 1a: own-half s^T (bf16) + 1b: remote half (fp8) ----
            with tc.tile_pool(name="ph1", bufs=1) as ph1:
                wspin_sb = ph1.tile([128, KT, D], BF16)
                wspin8_sb = ph1.tile([128, KT, D], F8)
                xtc = {}

                def load_chunk(nch, split_first=False):
                    if nch < 2:
                        t = ph1.tile([128, KT, 512], BF16, name=f"xtb{nch}",
                                     tag="xtb", bufs=2)
                        src = xtb_d
                        off = nch * 512
                    else:
                        t = ph1.tile([128, KT, 512], F8, name=f"xt8{nch}",
                                     tag="xt8", bufs=2)
                        src = xt8_d
                        off = (nch - 2) * 512
                    if split_first:
                        nc.sync.dma_start(
                            out=t[:, 0:1, :], in_=src[:, 0:1, off:off + 512])
                        nc.sync.dma_start(
                            out=t[:, 1:2, :], in_=src[:, 1:2, off:off + 512])
                        nc.sync.dma_start(
                            out=t[:, 2:8, :], in_=src[:, 2:8, off:off + 512])
                    else:
                        nc.sync.dma_start(out=t[:], in_=src[:, :, off:off + 512])
                    xtc[nch] = t

                # hot-path DMA order: first-pass operands only (~0.375MB)
                nc.sync.dma_start(out=wspin_sb[:, 0:1, :], in_=wspin_d[:, 0:1, :])
                load_chunk(0, split_first=True)
                nc.sync.dma_start(out=wspin_sb[:, 1:2, :], in_=wspin_d[:, 1:2, :])
                nc.sync.dma_start(out=wspin_sb[:, 2:4, :], in_=wspin_d[:, 2:4, :])
                nc.sync.dma_start(out=wspin_sb[:, 4:8, :], in_=wspin_d[:, 4:8, :])
                load_chunk(1)
                wj8_sb = wjp.tile([128, KT, D], F8)

                def st_write_own(ot, sl, ps, nch):
                    # bias add twice from PSUM: bf16 ST (+ mean accum on
                    # DVE) and fp8 ST8 (ACT Identity, AP bias)
                    nc.vector.tensor_scalar(
                        out=ST[:, ot, sl],
                        in0=ps[:],
                        scalar1=bspin_sb[:, ot:ot + 1],
                        scalar2=None,
                        op0=ADD, op1=ADD,
                        accum_out=mf4[:, ot, nch:nch + 1],
                    )
                    nc.scalar.activation(
                        out=ST8[:, ot, sl], in_=ps[:],
                        func=IDENT,
                        bias=bspin_sb[:, ot:ot + 1], scale=1.0,
                    )

                # chunk 0 in kt-split passes so matmuls start on ~0.75MB
                with tc.tile_pool(name="ps1a", bufs=1, space="PSUM") as ps1a:
                    ps_n0 = [ps1a.tile([128, 512], F32, name=f"psn0_{ot}",
                                       tag=f"psn0_{ot}")
                             for ot in range(KT)]
                    kt0 = 0
                    for pi, klen in enumerate((1, 1, 2, 4)):
                        for ot in range(KT):
                            for kt in range(kt0, kt0 + klen):
                                nc.tensor.matmul(
                                    ps_n0[ot][:],
                                    wspin_sb[:, kt, ot * 128:(ot + 1) * 128],
                                    xtc[0][:, kt, :],
                                    start=(kt == 0), stop=(kt == KT - 1),
                                )
                        kt0 += klen
                        if pi == 0:
                            # queue the rest of the input DMA behind the
                            # hot ones (wspin8 first: chunk-2's stationary)
                            nc.sync.dma_start(out=wspin8_sb[:], in_=wspin8_d[:])
                            load_chunk(2)
                            load_chunk(3)
                            nc.sync.dma_start(out=wj8_sb[:], in_=wj_d[:])
                    for ot in range(KT):
                        st_write_own(ot, slice(0, 512), ps_n0[ot], 0)

                with (
                    tc.tile_pool(name="ps1", bufs=3, space="PSUM") as ps1,
                    tc.tile_pool(name="ps3", bufs=2, space="PSUM") as ps3,
                ):
                    def transpose_mt(mt):
                        # all 8 d-tiles of one key tile into ONE psum bank,
                        # then single 1024-wide copies (latency-amortized).
                        # SN/SQ copies spread over phase-1/2 matmuls.
                        if mt < QT:
                            tp = ps3.tile([128, KT, 128], BF16, name="tpb",
                                          tag="tpb", bufs=2)
                            for j in range(KT):
                                nc.tensor.transpose(
                                    tp[:, j, :],
                                    ST[:, j, mt * 128:(mt + 1) * 128],
                                    ident_b[:],
                                )
                            if mt % 2 == 0:
                                nc.scalar.copy(SN[:, mt, :], tp[:])
                            else:
                                nc.vector.tensor_copy(SN[:, mt, :], tp[:])
                            nc.vector.tensor_scalar_mul(
                                SQ256[:, mt, :], tp[:], P_SCALE)
                        else:
                            # fp8 transposes land in PSUM at element step 2
                            tp = ps3.tile([128, KT, 128, 2], F8, name="tp8",
                                          tag="tp8", bufs=3)
                            for j in range(KT):
                                nc.tensor.transpose(
                                    tp[:, j, :, 0:1],
                                    ST8[:, j, mt * 128:(mt + 1) * 128],
                                    ident_8[:],
                                )
                            if mt % 2 == 0:
                                nc.scalar.copy(SN[:, mt, :], tp[:, :, :, 0:1])
                            else:
                                nc.vector.tensor_copy(
                                    SN[:, mt, :], tp[:, :, :, 0:1])

                    # chunk 1: own half, bf16
                    for ot in range(KT):
                        ps = ps1.tile([128, 512], F32)
                        for kt in range(KT):
                            nc.tensor.matmul(
                                ps[:],
                                wspin_sb[:, kt, ot * 128:(ot + 1) * 128],
                                xtc[1][:, kt, :],
                                start=(kt == 0), stop=(kt == KT - 1),
                            )
                        st_write_own(ot, slice(512, 1024), ps, 1)
                    for mt in range(0, 4):
                        transpose_mt(mt)
                    # chunks 2,3: remote half, fp8 DoubleRow; psum holds
                    # 32*(x@W.T); ACT rescales + bias + mean accum
                    for nch in (2, 3):
                        sl = slice(nch * 512, (nch + 1) * 512)
                        for ot in range(KT):
                            ps = ps1.tile([128, 512], F32)
                            for dp in range(KP):
                                nc.tensor.matmul(
                                    ps[:],
                                    wspin8_sb[:, 2 * dp:2 * dp + 2,
                                              ot * 128:(ot + 1) * 128],
                                    xtc[nch][:, 2 * dp:2 * dp + 2, :],
                                    start=(dp == 0), stop=(dp == KP - 1),
                                    perf_mode=DR,
                                )
                            nc.scalar.activation(
                                out=ST8[:, ot, sl], in_=ps[:],
                                func=IDENT,
                                bias=bspin_sb[:, ot:ot + 1],
                                scale=1.0 / WJ_SCALE,
                                accum_out=mf4[:, ot, nch:nch + 1],
                            )
                        if nch == 3:
                            # mean-field reduce ahead of the remaining
                            # transpose copies in the DVE queue so the g
                            # matmuls unblock early
                            for ot in range(KT):
                                nc.vector.reduce_sum(
                                    out=mf[:, ot:ot + 1], in_=mf4[:, ot, :],
                                    axis=mybir.AxisListType.X,
                                )
                            nc.vector.tensor_scalar_mul(
                                mfs[:, :, 0:1], mf[:], WJ_SCALE / N)
                        for mt in range(4 * (nch - 1), 4 * nch):
                            transpose_mt(mt)
                    for mt in range(12, MT):
                        transpose_mt(mt)

            # ---- Phase 2: EF^T = 32*W_J^T . s^T (fp8 DoubleRow); g ----
            EF8 = efp.tile([128, KT, NQ], F8)  # [d-in-tile, d-tile, query], 32x
            with (
                tc.tile_pool(name="ph2", bufs=1) as ph2,
                tc.tile_pool(name="ps4", bufs=4, space="PSUM") as ps4,
                tc.tile_pool(name="ps2", bufs=1, space="PSUM") as ps2,
            ):
                wglob_sb = ph2.tile([128, KT, D], F8)
                nc.sync.dma_start(out=wglob_sb[:], in_=wglob_d[:])

                # g^T = mf^T . W_global^T, fp8 DoubleRow, both sides
                # pre-scaled x32; psum holds 1024*g
                gps = ps2.tile([1, 2, 512], F32)
                for ch in range(2):
                    for dp in range(KP):
                        nc.tensor.matmul(
                            gps[:, ch, :],
                            mfs[:, 2 * dp:2 * dp + 2, 0:1],
                            wglob_sb[:, 2 * dp:2 * dp + 2,
                                     ch * 512:(ch + 1) * 512],
                            start=(dp == 0), stop=(dp == KP - 1),
                            perf_mode=DR,
                        )
                nc.vector.tensor_scalar_mul(
                    gT[:], gps[0:1, :, :].rearrange("p a b -> p (a b)"),
                    1.0 / (WJ_SCALE * WJ_SCALE),
                )

                for ch in range(2):
                    for ot in range(KT):
                        ps = ps4.tile([128, 512], F32)
                        for dp in range(KP):
                            nc.tensor.matmul(
                                ps[:],
                                wj8_sb[:, 2 * dp:2 * dp + 2,
                                       ot * 128:(ot + 1) * 128],
                                ST8[:, 2 * dp:2 * dp + 2,
                                    ch * 512:(ch + 1) * 512],
                                start=(dp == 0), stop=(dp == KP - 1),
                                perf_mode=DR,
                            )
                        nc.vector.tensor_copy(
                            EF8[:, ot, ch * 512:(ch + 1) * 512], ps[:]
                        )
            wjp_cm.__exit__(None, None, None)

            # ---- Phase 3: grouped attention ----
            with tc.tile_pool(name="att_sn", bufs=1) as att_sn:
                with tc.tile_pool(name="work", bufs=2) as work:
                    psA_cm = tc.tile_pool(name="psA", bufs=2, space="PSUM")
                    psA = psA_cm.__enter__()
                    psB_cm = tc.tile_pool(name="psB", bufs=2, space="PSUM")
                    psB = psB_cm.__enter__()
                    ps5l_cm = tc.tile_pool(name="ps5l", bufs=2, space="PSUM")
                    ps5l = ps5l_cm.__enter__()

                    def att_group(grp):
                        q0 = grp * 512
                        # A: scores^T[m, q] for all 2048 keys x 512 queries
                        P8T = work.tile([128, MT, 512], F8, name="P8T",
                                        tag="P8T", bufs=2)
                        for mt in range(MT):
                            sc_ps = psA.tile([128, 512], F32)
                            for dp in range(KP):
                                nc.tensor.matmul(
                                    sc_ps[:],
                                    ST8[:, 2 * dp:2 * dp + 2,
                                        mt * 128:(mt + 1) * 128],
                                    EF8[:, 2 * dp:2 * dp + 2, q0:q0 + 512],
                                    start=(dp == 0), stop=(dp == KP - 1),
                                    perf_mode=DR,
                                )
                            # no max subtraction: |scores|*SCALE < ~2 here.
                            # EF8 carries 32x, so fold 1/32 into the scale.
                            nc.scalar.activation(
                                out=P8T[:, mt, :], in_=sc_ps[:],
                                func=mybir.ActivationFunctionType.Exp,
                                bias=0.0, scale=float(SCALE / WJ_SCALE),
                            )
                        # B: rowsums over keys via ones DoubleRow matmuls
                        rs_ps = psB.tile([1, 512], F32)
                        for mp in range(MT // 2):
                            nc.tensor.matmul(
                                rs_ps[:],
                                ones8[:, :, 0:1],
                                P8T[:, 2 * mp:2 * mp + 2, :],
                                start=(mp == 0), stop=(mp == MT // 2 - 1),
                                perf_mode=DR,
                            )
                        rs_row = stats.tile([1, 512], BF16)
                        nc.vector.tensor_copy(rs_row[:], rs_ps[:])
                        # rs/256 as columns (rank-1 transpose matmuls), then
                        # reciprocal -> 256/rs per query partition
                        rsc_ps = psB.tile([128, 4], F32, name="rsc", tag="rsc",
                                          bufs=2)
                        rinv256 = stats.tile([128, 4], F32)
                        for j in range(4):
                            nc.tensor.matmul(
                                rsc_ps[:, j:j + 1],
                                rs_row[0:1, j * 128:(j + 1) * 128],
                                c_inv256[:],
                                start=True, stop=True,
                            )
                        nc.vector.reciprocal(rinv256[:], rsc_ps[:])
                        return P8T, rs_row, rinv256

                    def lf_tile(grp, qtl, P8T, rs_row, rinv256):
                        qt = grp * 4 + qtl
                        qoff = qtl * 128
                        nsl = 1
                        for dch in range(2):
                            dsl = slice(dch * 512, (dch + 1) * 512)
                            plf = ps5l.tile([128, 512], F32)
                            for mp in range(MT // 2):
                                nc.tensor.matmul(
                                    plf[:],
                                    P8T[:, 2 * mp:2 * mp + 2, qoff:qoff + 128],
                                    SN[:, 2 * mp:2 * mp + 2, dsl],
                                    start=(mp == 0), stop=False,
                                    perf_mode=DR,
                                )
                            # g term last (waits on the DVE rs_row copy):
                            # rowsum[q] x g[d]; the rinv multiply below
                            # turns it into exactly g
                            nc.tensor.matmul(
                                plf[:], rs_row[0:1, qoff:qoff + 128], gT[:, dsl],
                                start=False, stop=True,
                            )
                            w = 512 // nsl
                            for si in range(nsl):
                                ssl = slice(si * w, (si + 1) * w)
                                osl = slice(dch * 512 + si * w,
                                            dch * 512 + (si + 1) * w)
                                z1 = work.tile([128, w], F32, name=f"z1_{w}",
                                               tag=f"z1_{w}", bufs=3)
                                nc.vector.tensor_scalar_mul(
                                    z1[:], plf[:, ssl],
                                    rinv256[:, qtl:qtl + 1])
                                z = work.tile([128, w], F32, name=f"z_{w}",
                                              tag=f"z_{w}", bufs=3)
                                nc.vector.tensor_add(
                                    z[:], z1[:], SQ256[:, qt, osl])
                                osb = work.tile([128, w], BF16, name=f"osb{w}",
                                                tag=f"osb{w}", bufs=4)
                                nc.scalar.activation(
                                    out=osb[:], in_=z[:],
                                    func=mybir.ActivationFunctionType.Tanh,
                                    bias=0.0, scale=beta256[:],
                                )
                                nc.sync.dma_start(
                                    out=out_d[qt * 128:(qt + 1) * 128, osl],
                                    in_=osb[:])

                    g0 = att_group(0)
                    for qtl in range(4):
                        lf_tile(0, qtl, *g0)
                    g1 = att_group(1)
                    for qtl in range(4):
                        lf_tile(1, qtl, *g1)
                    ps5l_cm.__exit__(None, None, None)
                    psB_cm.__exit__(None, None, None)
                    psA_cm.__exit__(None, None, None)

            efp_cm.__exit__(None, None, None)

    nc.compile()
    nc.m = get_hw_module(nc.m)
    return nc


def _tile_kxm(a, np_dt):
    """(K, M) row-major -> [128, K//128, M] with k = kt*128 + p."""
    k, m = a.shape
    return np.ascontiguousarray(
        a.reshape(k // 128, 128, m).transpose(1, 0, 2)
    ).astype(np_dt)


def kernel(x, W_spin, b_spin, W_global, W_J, beta):
    global LAST_RESULT
    x = np.asarray(x, dtype=np.float32)
    W_spin = np.asarray(W_spin, dtype=np.float32)
    b_spin = np.asarray(b_spin, dtype=np.float32)
    W_global = np.asarray(W_global, dtype=np.float32)
    W_J = np.asarray(W_J, dtype=np.float32)
    beta = np.asarray(beta, dtype=np.float32)

    if "nc" not in _CACHED:
        _CACHED["nc"] = _build()
    nc = _CACHED["nc"]

    BF = ml_dtypes.bfloat16
    E4 = ml_dtypes.float8_e4m3
    wspinT = _tile_kxm(W_spin.T, BF)              # W_spin.T is (k, o)
    wspin8T = _tile_kxm(W_spin.T * WJ_SCALE, E4)
    wjT = _tile_kxm(W_J.T * WJ_SCALE, E4)
    wglobT = _tile_kxm(W_global.T * WJ_SCALE, E4)
    bspin = np.ascontiguousarray(b_spin.reshape(KT, 128).T).astype(np.float32)
    beta_h = beta.reshape(1, 1).astype(np.float32)

    in_maps = []
    for core in range(8):
        b, h = divmod(core, 2)
        xb = x[b]
        own = xb[h * NQ:(h + 1) * NQ]
        rem = xb[(1 - h) * NQ:(2 - h) * NQ]
        xtb = _tile_kxm(np.ascontiguousarray(own.T), BF)   # (k=D, n=NQ)
        xt8 = _tile_kxm(np.ascontiguousarray(rem.T), E4)
        in_maps.append({
            "xtb": xtb, "xt8": xt8, "wspinT": wspinT, "wspin8T": wspin8T,
            "wjT": wjT, "wglobT": wglobT, "bspin": bspin, "beta": beta_h,
        })

    LAST_RESULT = bass_utils.run_bass_kernel_spmd(
        nc, in_maps, core_ids=list(range(8))
    )

    out = np.empty((B, N, D), dtype=np.float32)
    for core in range(8):
        b, h = divmod(core, 2)
        out[b, h * NQ:(h + 1) * NQ, :] = (
            LAST_RESULT.results[core]["out"].astype(np.float32))
    return out
